# revision 21
# baseline (speedup 1.0000x reference)
"""Trainium2 Bass kernel for CalculateInstanceSize (segment_reduce).

Contract: kernel(seg_outs, pad_ins_outs) -> [B, N, 3] float32, matching
the jax reference. B=8 batches are data-parallel across the 8 NeuronCores;
each core computes its batch's per-row regression (unit length) and the
three weighted reductions over pad [N, H, W].

v2 layout notes (vs the fp16 baseline):
- pad ships as fp8e4 (halves HBM traffic vs fp16; the per-element rounding
  is random so the big sums keep ~1e-4 relative error).
- The weighted reductions run as DoubleRow fp8 matmuls (K=256 per
  instruction) with on-device dynamically scaled hi/lo-split fp8 weights:
  w*s = hi + lo with both parts fp8e4, so the systematic weight-rounding
  error is ~2^-8 instead of fp8's 2^-4. PSUM rows: [T_hi, T_lo, I_hi, I_lo].
- horizontal needs max_w(T_hi+T_lo): a [K=2] ones-matmul re-reads the
  evacuated PSUM pair and lands T for 8 instances in one [8, W] PSUM bank
  (slot one-hot lhsT), so the W-max runs 8 instances per DVE reduce.
- instance = sum_w(I_hi+I_lo) comes from the ACT evacuation's accum and is
  combined at the end (partitions 2,3 -> DMA realign, engines can't read
  partition offsets on walrus).
- occ row-counts are spread over DVE/ACT/Pool; occ = count > 0 for all
  three formulations.
- pad DMAs in groups of 4 instances (1 MiB) for bandwidth.
"""

import sys

sys.path.insert(0, "/opt/trn_rl_repo")

import numpy as np

import concourse.bass as bass
import concourse.tile as tile
from concourse import bacc, bass_isa, mybir
from concourse.bass_utils import run_bass_kernel_spmd

F32 = mybir.dt.float32
F16 = mybir.dt.float16
BF16 = mybir.dt.bfloat16
FP8 = mybir.dt.float8e4
AX = mybir.AxisListType
OP = mybir.AluOpType
ACTF = mybir.ActivationFunctionType
PERF = mybir.MatmulPerfMode

B, H, W, N = 8, 512, 512, 32
NCH = H // 128  # h-chunks of 128 partitions
GN = 4  # instances per pad DMA group
ROAD = 3.25


def build_kernel(reps: int = 1, probe: frozenset = frozenset()):
    import os

    if not probe and os.environ.get("BASS_PROBE"):
        probe = frozenset(os.environ["BASS_PROBE"].split(","))
    nc = bacc.Bacc("TRN2", target_bir_lowering=False, debug=False, num_devices=B)

    seg = nc.dram_tensor("seg", [128, NCH, W], BF16, kind="ExternalInput").ap()
    pad = nc.dram_tensor("pad", [N, H, W], FP8, kind="ExternalInput").ap()
    yf = nc.dram_tensor("yf", [128, NCH], F32, kind="ExternalInput").ap()
    tril = nc.dram_tensor("tril", [128, 128], F32, kind="ExternalInput").ap()
    amin4 = nc.dram_tensor("amin4", [128, NCH, W], F16, kind="ExternalInput").ap()
    amax4 = nc.dram_tensor("amax4", [128, NCH, W], F16, kind="ExternalInput").ap()
    out = nc.dram_tensor("out", [3, N], F32, kind="ExternalOutput").ap()

    with tile.TileContext(nc) as tc:
        emit(tc, out, seg, pad, yf, tril, amin4, amax4, reps, probe)
    nc.compile()
    return nc


def emit(tc, out, seg, pad, yf, tril, amin4, amax4, reps=1, probe=frozenset()):
    nc = tc.nc
    import contextlib

    ctx = contextlib.ExitStack()
    with ctx:
        consts = ctx.enter_context(tc.tile_pool(name="consts", bufs=1))
        padp = ctx.enter_context(tc.tile_pool(name="padp", bufs=3))
        evacp = ctx.enter_context(tc.tile_pool(name="evacp", bufs=34))
        jhp = ctx.enter_context(tc.tile_pool(name="jhp", bufs=4))
        psp = ctx.enter_context(tc.psum_pool(name="psp", bufs=4))
        php = ctx.enter_context(tc.psum_pool(name="php", bufs=1))
        psv = ctx.enter_context(tc.psum_pool(name="psv", bufs=1))
        pss = ctx.enter_context(tc.psum_pool(name="pss", bufs=1))

        # ---- prologue inputs (seg first: it heads the critical path) ----
        SEGB = consts.tile([128, NCH, W], BF16)
        nc.sync.dma_start(SEGB[:], seg[:])
        AMIN4 = consts.tile([128, NCH, W], F16)
        nc.sync.dma_start(AMIN4[:], amin4[:])
        AMAX4 = consts.tile([128, NCH, W], F16)
        nc.sync.dma_start(AMAX4[:], amax4[:])
        YF = consts.tile([128, NCH], F32)
        nc.sync.dma_start(YF[:], yf[:])
        TRIL = consts.tile([128, 128], F32)
        nc.sync.dma_start(TRIL[:], tril[:])
        ONES1 = consts.tile([128, 1], F32)
        nc.gpsimd.memset(ONES1[:], 1.0)
        NEGH = consts.tile([128, 1], F32)
        nc.gpsimd.memset(NEGH[:], -0.5)

        # ---- per-row x_min / x_max in column space ----
        # mask = seg > 0; R0 = max_w (W-w)*m -> xmin = W - R0
        #                 R1 = max_w (w+1)*m -> xmax = R1 - 1
        MSK = consts.tile([128, NCH, W], F16)
        nc.vector.tensor_scalar(
            out=MSK[:], in0=SEGB[:], scalar1=0.0, scalar2=None, op0=OP.is_gt
        )
        TMIN = consts.tile([128, NCH, W], F16)
        nc.vector.tensor_tensor(out=TMIN[:], in0=MSK[:], in1=AMIN4[:], op=OP.mult)
        TMAX = consts.tile([128, NCH, W], F16)
        nc.vector.tensor_tensor(out=TMAX[:], in0=MSK[:], in1=AMAX4[:], op=OP.mult)
        R0 = consts.tile([128, NCH], F32)
        nc.vector.tensor_reduce(out=R0[:], in_=TMIN[:], axis=AX.X, op=OP.max)
        R1 = consts.tile([128, NCH], F32)
        nc.vector.tensor_reduce(out=R1[:], in_=TMAX[:], axis=AX.X, op=OP.max)
        XMIN4 = consts.tile([128, NCH], F32)
        nc.vector.tensor_scalar(
            out=XMIN4[:], in0=R0[:], scalar1=-1.0, scalar2=float(W), op0=OP.mult,
            op1=OP.add,
        )
        XMAX4 = consts.tile([128, NCH], F32)
        nc.vector.tensor_scalar(
            out=XMAX4[:], in0=R1[:], scalar1=1.0, scalar2=None, op0=OP.subtract
        )

        # ---- validity + rank (global h-cumsum via triangular matmul) ----
        NE4 = consts.tile([128, NCH], F32)
        nc.vector.tensor_tensor(out=NE4[:], in0=XMIN4[:], in1=XMAX4[:], op=OP.not_equal)
        V4 = consts.tile([128, NCH], F32)
        nc.vector.scalar_tensor_tensor(
            out=V4[:], in0=XMAX4[:], scalar=-0.5, in1=NE4[:], op0=OP.is_gt, op1=OP.mult
        )
        CUM4 = pss.tile([128, NCH], F32, tag="cum4")
        nc.tensor.matmul(out=CUM4[:], lhsT=TRIL[:], rhs=V4[:], start=True, stop=True)
        CS = pss.tile([1, NCH], F32, tag="small")
        nc.tensor.matmul(out=CS[:], lhsT=ONES1[:], rhs=V4[:], start=True, stop=True)
        # exclusive prefix of per-column sums
        OFFS = consts.tile([1, NCH], F32)
        nc.vector.memset(OFFS[:], 0.0)
        nc.vector.tensor_copy(OFFS[0:1, 1:NCH], CS[0:1, 0 : NCH - 1])
        nc.vector.tensor_tensor(
            out=OFFS[0:1, 2:NCH], in0=OFFS[0:1, 2:NCH], in1=OFFS[0:1, 0 : NCH - 2],
            op=OP.add,
        )
        # scalars packed into SCP = [t, t-1, n_valid, 0]
        SCP = consts.tile([1, NCH], F32)
        NV = SCP[0:1, 2:3]
        nc.vector.tensor_reduce(out=NV, in_=CS[:], axis=AX.X, op=OP.add)
        TVv = SCP[0:1, 0:1]
        nc.vector.tensor_scalar(
            out=TVv, in0=NV, scalar1=0.15, scalar2=None, op0=OP.mult
        )
        nc.vector.tensor_scalar(
            out=SCP[0:1, 1:2], in0=TVv, scalar1=1.0, scalar2=None, op0=OP.subtract
        )
        nc.vector.memset(SCP[0:1, 3:4], 0.0)
        SCB = consts.tile([128, NCH], F32)
        nc.gpsimd.partition_broadcast(SCB[:], SCP[0:1, :])
        OFFSB = consts.tile([128, NCH], F32)
        nc.gpsimd.partition_broadcast(OFFSB[:], OFFS[0:1, :])
        RANK4 = consts.tile([128, NCH], F32)
        nc.vector.scalar_tensor_tensor(
            out=RANK4[:], in0=CUM4[:], scalar=-1.0, in1=OFFSB[:], op0=OP.add,
            op1=OP.add,
        )
        # keep = valid & rank>t-1 & rank>=1 & (n-rank)>t & (n-rank)>1.5
        M4 = consts.tile([128, NCH], F32)
        nc.vector.tensor_scalar(
            out=M4[:], in0=RANK4[:], scalar1=SCB[:, 2:3], scalar2=-1.0,
            op0=OP.subtract, op1=OP.mult,
        )
        K1 = consts.tile([128, NCH], F32)
        nc.vector.scalar_tensor_tensor(
            out=K1[:], in0=RANK4[:], scalar=SCB[:, 1:2], in1=V4[:], op0=OP.is_gt,
            op1=OP.mult,
        )
        K2 = consts.tile([128, NCH], F32)
        nc.vector.scalar_tensor_tensor(
            out=K2[:], in0=RANK4[:], scalar=0.5, in1=K1[:], op0=OP.is_gt, op1=OP.mult
        )
        K3 = consts.tile([128, NCH], F32)
        nc.vector.scalar_tensor_tensor(
            out=K3[:], in0=M4[:], scalar=SCB[:, 0:1], in1=K2[:], op0=OP.is_gt,
            op1=OP.mult,
        )
        W4 = consts.tile([128, NCH], F32)
        nc.vector.scalar_tensor_tensor(
            out=W4[:], in0=M4[:], scalar=1.5, in1=K3[:], op0=OP.is_gt, op1=OP.mult
        )

        # ---- weighted sums S = [Sw, Sy, Syy, SxL, SxyL, SxR, SxyR] ----
        # (ones-matmul over the h-partitions; all addends here are integers
        # so the PE's decomposed fp32 multiply is exact)
        S7 = consts.tile([128, NCH, 7], F32)
        nc.vector.tensor_copy(S7[:, :, 0], W4[:])
        nc.vector.tensor_tensor(out=S7[:, :, 1], in0=W4[:], in1=YF[:], op=OP.mult)
        nc.vector.tensor_tensor(out=S7[:, :, 2], in0=S7[:, :, 1], in1=YF[:], op=OP.mult)
        nc.vector.tensor_tensor(out=S7[:, :, 3], in0=W4[:], in1=XMIN4[:], op=OP.mult)
        nc.vector.tensor_tensor(out=S7[:, :, 4], in0=S7[:, :, 3], in1=YF[:], op=OP.mult)
        nc.vector.tensor_tensor(out=S7[:, :, 5], in0=W4[:], in1=XMAX4[:], op=OP.mult)
        nc.vector.tensor_tensor(out=S7[:, :, 6], in0=S7[:, :, 5], in1=YF[:], op=OP.mult)
        SS = pss.tile([1, 7], F32, tag="small")
        for c in range(NCH):
            nc.tensor.matmul(
                out=SS[:], lhsT=ONES1[:], rhs=S7[:, c, :], start=(c == 0),
                stop=(c == NCH - 1),
            )

        # ---- 2x2 normal-equation solve, batched on [1,k] rows ----
        # G pairs (even*odd): (0,1)=(Sw*SxyL, Sy*SxL)  (2,3)=(Syy*SxL, Sy*SxyL)
        #                     (4,5)=(Sw*SxyR, Sy*SxR)  (6,7)=(Syy*SxR, Sy*SxyR)
        #                     (8,9)=(Syy*Sw, Sy*Sy)
        # D[0:5] = G[even] - G[odd] = [nsL, niL, nsR, niR, det]
        G = consts.tile([1, 10], F32)
        SR = consts.tile([1, 7], F32)
        nc.vector.tensor_copy(SR[:], SS[:])  # PSUM -> SBUF (TT can't read 2x PSUM)

        # strided pair products out of the [1,7] sums row
        def pair(dst0, a0, a1):
            nc.vector.tensor_tensor(
                out=G[0:1, dst0 : dst0 + 2], in0=a0, in1=a1, op=OP.mult
            )

        up01 = SR[0:1, 0:2]  # (Sw, Sy)
        dn21 = SR[0:1, 2:0:-1]  # (Syy, Sy)
        pair(0, up01, SR[0:1, 4:2:-1])  # (Sw*SxyL, Sy*SxL)
        pair(2, dn21, SR[0:1, 3:5])  # (Syy*SxL, Sy*SxyL)
        pair(4, up01, SR[0:1, 6:4:-1])  # (Sw*SxyR, Sy*SxR)
        pair(6, dn21, SR[0:1, 5:7])  # (Syy*SxR, Sy*SxyR)
        pair(8, dn21, up01)  # (Syy*Sw, Sy*Sy)
        D = consts.tile([1, 8], F32)
        nc.vector.tensor_tensor(
            out=D[0:1, 0:5], in0=G[0:1, 0:10:2], in1=G[0:1, 1:10:2], op=OP.subtract
        )
        DET = D[0:1, 4:5]
        OKV = D[0:1, 5:6]
        nc.vector.tensor_scalar(
            out=OKV, in0=DET, scalar1=0.0, scalar2=None, op0=OP.is_gt
        )
        # safe = det*ok + (1-ok); rsafe = 1/safe
        SAFE = D[0:1, 6:7]
        nc.vector.scalar_tensor_tensor(
            out=SAFE, in0=DET, scalar=1.0, in1=OKV, op0=OP.subtract, op1=OP.mult
        )  # (det-1)*ok
        nc.vector.tensor_scalar(
            out=SAFE, in0=SAFE, scalar1=1.0, scalar2=None, op0=OP.add
        )  # (det-1)*ok + 1 = det*ok + (1-ok)
        RS = D[0:1, 7:8]
        nc.vector.reciprocal(out=RS, in_=SAFE)
        SLIC = consts.tile([1, NCH], F32)
        nc.vector.tensor_scalar(
            out=SLIC[:], in0=D[0:1, 0:4], scalar1=RS, scalar2=OKV, op0=OP.mult,
            op1=OP.mult,
        )

        # ---- unit / unit^2 weights ----
        SB = consts.tile([128, NCH], F32)
        nc.gpsimd.partition_broadcast(SB[:], SLIC[0:1, :])
        PRL = consts.tile([128, NCH], F32)
        nc.vector.tensor_scalar(
            out=PRL[:], in0=YF[:], scalar1=SB[:, 0:1], scalar2=SB[:, 1:2],
            op0=OP.mult, op1=OP.add,
        )
        PRR = consts.tile([128, NCH], F32)
        nc.vector.tensor_scalar(
            out=PRR[:], in0=YF[:], scalar1=SB[:, 2:3], scalar2=SB[:, 3:4],
            op0=OP.mult, op1=OP.add,
        )
        WID = consts.tile([128, NCH], F32)
        nc.vector.tensor_tensor(out=WID[:], in0=PRR[:], in1=PRL[:], op=OP.subtract)
        nc.vector.tensor_scalar(
            out=WID[:], in0=WID[:], scalar1=1.0, scalar2=None, op0=OP.max
        )
        RCP = consts.tile([128, NCH], F32)
        nc.vector.reciprocal(out=RCP[:], in_=WID[:])
        UU = consts.tile([128, NCH, 2], F32)
        nc.vector.tensor_scalar(
            out=UU[:, :, 0], in0=RCP[:], scalar1=ROAD, scalar2=None, op0=OP.mult
        )
        nc.vector.scalar_tensor_tensor(
            out=UU[:, :, 1], in0=RCP[:], scalar=ROAD * ROAD, in1=RCP[:],
            op0=OP.mult, op1=OP.mult,
        )

        # ---- dynamic scales + hi/lo fp8 weight split ----
        # m1 = max_h unit (all partitions), s1 = 128/m1, s2 = 128/m1^2.
        MR = consts.tile([128, 1], F32)
        nc.vector.tensor_reduce(out=MR[:], in_=UU[:, :, 0], axis=AX.X, op=OP.max)
        MRA = consts.tile([128, 1], F32)
        nc.gpsimd.partition_all_reduce(
            MRA[:], MR[:], channels=128, reduce_op=bass_isa.ReduceOp.max
        )
        # SCL columns: 0 = s1, 1 = s2, 2 = 1/s1, 3 = 1/s2
        SCL = consts.tile([128, 4], F32)
        nc.vector.tensor_scalar(
            out=SCL[:, 2:3], in0=MRA[:], scalar1=1.0 / 128.0, scalar2=None,
            op0=OP.mult,
        )
        nc.vector.reciprocal(out=SCL[:, 0:1], in_=SCL[:, 2:3])
        nc.vector.scalar_tensor_tensor(
            out=SCL[:, 1:2], in0=SCL[:, 0:1], scalar=1.0 / 128.0, in1=SCL[:, 0:1],
            op0=OP.mult, op1=OP.mult,
        )
        nc.vector.reciprocal(out=SCL[:, 3:4], in_=SCL[:, 1:2])
        # scaled weights WS[:, :, 0] = unit*s1, WS[:, :, 1] = unit2*s2
        WS = consts.tile([128, NCH, 2], F32)
        nc.vector.tensor_scalar(
            out=WS[:, :, 0], in0=UU[:, :, 0], scalar1=SCL[:, 0:1], scalar2=None,
            op0=OP.mult,
        )
        nc.vector.tensor_scalar(
            out=WS[:, :, 1], in0=UU[:, :, 1], scalar1=SCL[:, 1:2], scalar2=None,
            op0=OP.mult,
        )
        # UU8[p, t, cp, m]: weight m for chunk c = 2*cp + t (t-step 16 B keeps
        # the DoubleRow weight-AP stride constraint).
        # m: 0 = u1hi, 1 = u1lo, 2 = u2hi, 3 = u2lo
        UU8 = consts.tile([128, 2, 2, 8], FP8)  # m padded to 8 -> t-step 16 B
        REM = consts.tile([128, NCH, 2], F32)
        for m in range(2):  # m-pair index: 0 -> (u1hi,u1lo) slots 0,1; 1 -> 2,3
            for c in range(NCH):
                cp, t = c // 2, c % 2
                hi = UU8[:, t, cp, 2 * m : 2 * m + 1]
                lo = UU8[:, t, cp, 2 * m + 1 : 2 * m + 2]
                src = WS[:, c, m : m + 1]
                rem = REM[:, c, m : m + 1]
                nc.vector.tensor_copy(hi, src)
                nc.vector.tensor_tensor(out=rem, in0=src, in1=hi, op=OP.subtract)
                nc.vector.tensor_copy(lo, rem)
        # slot one-hot lhsT for the T-combine ones-matmul: E32[k, n, :] has
        # ones only in column n (k = 0, 1 over the PSUM hi/lo partitions),
        # so instance n's T row lands on partition n of the shared PSUM bank
        E32 = consts.tile([2, N, N], BF16)
        nc.vector.memset(E32[:], 0.0)
        for j in range(N):
            nc.vector.memset(E32[:, j, j : j + 1], 1.0)

        padr = pad.rearrange("n (c p) w -> p n c w", p=128)

        # ---- main loop over instances ----
        do_dma = "nodma" not in probe
        do_mm = "nomm" not in probe
        do_cmp = "nocmp" not in probe
        do_evac = do_mm and "noevac" not in probe
        # junk compare outputs, shared across reps
        JD16 = consts.tile([128, W // 2], F16)
        JD8 = consts.tile([128, W // 2], F16)
        JA8 = consts.tile([128, W // 2], F16)
        JAF = consts.tile([128, W], F16)
        T16 = 0.15619  # fp16-bits threshold: packed hi byte > 0x30 (fp8 0.5)
        for _rep in range(reps):
            # CNT: even-w counts (or full counts), CNTB: odd-w counts
            CNT = consts.tile([128, NCH, N], F32)
            CNTB = consts.tile([128, NCH, N], F32)
            if not do_cmp:
                nc.vector.memset(CNT[:], 1.0)
                nc.vector.memset(CNTB[:], 0.0)
            INST32 = consts.tile([N, N], F32)  # rows 0-3 = sum_w of the PSUM rows
            nc.gpsimd.memset(INST32[:], 0.0)
            # compare modes per chunk:
            #  M1: odd pads via fp16 view on DVE (2x) + even pads (stride-2
            #      fp8) on DVE
            #  M2: odd pads via fp16 view on DVE (2x) + even pads on ACT
            #  A : full-width fp8 relu on ACT
            shares = {"M1": 62, "M2": 66, "A": 0}
            assign, used = [], {k: 0 for k in shares}
            for i in range(N * NCH):
                k = max(shares, key=lambda e: (i + 1) * shares[e] / 128 - used[e])
                used[k] += 1
                assign.append(k)
            PH32 = None
            if do_evac:
                PH32 = php.tile([N, W], F32, tag="ph")
            pair_tiles = []
            for n in range(N):
                g, i = divmod(n, GN)
                if i == 0:
                    PT4 = padp.tile([128, GN, NCH, W], FP8, tag="pt")
                    if do_dma or (g < 3 and _rep == 0):
                        nc.sync.dma_start(
                            PT4[:], padr[:, g * GN : (g + 1) * GN, :, :]
                        )
                PS4 = None
                if do_mm:
                    PS4 = psp.tile([4, W], F32, tag="ps")
                for cp in range(NCH // 2) if do_mm else []:
                    nc.tensor.matmul(
                        out=PS4[:],
                        lhsT=UU8[:, :, cp, 0:4],
                        rhs=PT4[:, i, 2 * cp : 2 * cp + 2, :],
                        start=(cp == 0),
                        stop=(cp == NCH // 2 - 1),
                        perf_mode=PERF.DoubleRow,
                    )
                for c in range(NCH) if do_cmp else []:
                    eng = assign[n * NCH + c]
                    chunk = PT4[:, i, c, :]
                    if eng == "A":
                        nc.scalar.activation(
                            out=JAF[:], in_=chunk, func=ACTF.Relu,
                            bias=NEGH[:, 0:1], scale=1.0,
                            accum_out=CNT[:, c, n : n + 1],
                        )
                        nc.vector.memset(CNTB[:, c, n : n + 1], 0.0)
                        continue
                    # odd pads: packed-fp16 view, hi byte > 0x30 <=> pad > 0.5
                    nc.vector.tensor_scalar(
                        out=JD16[:], in0=chunk.bitcast(F16), scalar1=T16,
                        scalar2=None, op0=OP.is_gt, op1=OP.add,
                        accum_out=CNTB[:, c, n : n + 1],
                    )
                    # even pads: stride-2 fp8
                    ev = PT4[:, i, c, 0:W:2]
                    if eng == "M1":
                        nc.vector.tensor_scalar(
                            out=JD8[:], in0=ev, scalar1=0.5, scalar2=None,
                            op0=OP.is_gt, op1=OP.add,
                            accum_out=CNT[:, c, n : n + 1],
                        )
                    else:
                        nc.scalar.activation(
                            out=JA8[:], in_=ev, func=ACTF.Relu,
                            bias=NEGH[:, 0:1], scale=1.0,
                            accum_out=CNT[:, c, n : n + 1],
                        )
                if do_evac:
                    PAIR4 = evacp.tile([4, W], BF16, tag="pair")
                    nc.scalar.activation(
                        out=PAIR4[:], in_=PS4[:], func=ACTF.Copy,
                        accum_out=INST32[0:4, n : n + 1],
                    )
                    pair_tiles.append(PAIR4)
            # T rows for all instances -> one [32, W] PSUM bank, emitted as a
            # single PE burst so the PE never stalls on ACT mid-loop
            if do_evac:
                for n in range(N):
                    nc.tensor.matmul(
                        out=PH32[:], lhsT=E32[:, n, :], rhs=pair_tiles[n][0:2, :],
                        start=(n == 0), stop=(n == N - 1),
                    )
            # one batched W-max for all instances: [32, W] -> [32, 1]
            HORC = consts.tile([N, 1], F32)
            if do_evac:
                nc.vector.tensor_reduce(
                    out=HORC[:], in_=PH32[:], axis=AX.X, op=OP.max
                )
            else:
                nc.vector.memset(HORC[:], 0.0)

            # ---- vertical: occ = cnt_even + cnt_odd > 0 ; vert = sum_h unit*occ
            OCC = consts.tile([128, NCH, N], F32)
            VERT = psv.tile([1, N], F32)
            nc.vector.tensor_tensor(out=OCC[:], in0=CNT[:], in1=CNTB[:], op=OP.add)
            nc.vector.tensor_scalar(
                out=OCC[:], in0=OCC[:], scalar1=0.0, scalar2=None, op0=OP.is_gt
            )
            for c in range(NCH):
                nc.tensor.matmul(
                    out=VERT[:],
                    lhsT=UU[:, c, 0:1],
                    rhs=OCC[:, c, :],
                    start=(c == 0),
                    stop=(c == NCH - 1),
                )

            VERTS = consts.tile([1, N], F32)
            nc.scalar.copy(out=VERTS[:], in_=VERT[:])
            # instance = (I_hi + I_lo) / s2: transpose INST32 so n is the
            # partition axis, then combine columns 2+3
            TI32 = consts.tile([N, N], F32)
            nc.vector.transpose(out=TI32[:], in_=INST32[:])
            OUTT = consts.tile([N, 2], F32)
            nc.vector.tensor_tensor(
                out=OUTT[:, 0:1], in0=TI32[:, 2:3], in1=TI32[:, 3:4], op=OP.add
            )
            nc.vector.tensor_scalar(
                out=OUTT[:, 0:1], in0=OUTT[:, 0:1], scalar1=SCL[0:N, 3:4],
                scalar2=None, op0=OP.mult,
            )
            # horizontal = HORC / s1
            nc.vector.tensor_scalar(
                out=OUTT[:, 1:2], in0=HORC[:], scalar1=SCL[0:N, 2:3],
                scalar2=None, op0=OP.mult,
            )
            nc.sync.dma_start(out[0:2, :].rearrange("r n -> n r"), OUTT[:])
            nc.sync.dma_start(out[2:3, :], VERTS[:])


_NC = None


def _get_nc():
    global _NC
    if _NC is None:
        _NC = build_kernel()
    return _NC


def _consts():
    yf = (
        np.arange(128, dtype=np.float32)[:, None]
        + 128.0 * np.arange(NCH, dtype=np.float32)[None, :]
    ).copy()
    tril = np.triu(np.ones((128, 128), dtype=np.float32))  # [k,m] = 1 iff k<=m
    wv = np.arange(W, dtype=np.float32)
    amin4 = np.broadcast_to((W - wv).astype(np.float16), (128, NCH, W)).copy()
    amax4 = np.broadcast_to((wv + 1.0).astype(np.float16), (128, NCH, W)).copy()
    return yf, tril, amin4, amax4


def make_in_maps(seg_outs: np.ndarray, pad_ins_outs: np.ndarray):
    import ml_dtypes

    yf, tril, amin4, amax4 = _consts()
    in_maps = []
    for b in range(B):
        seg_b = (
            seg_outs[b, :, :, 1]
            .reshape(NCH, 128, W)
            .transpose(1, 0, 2)
            .astype(ml_dtypes.bfloat16)
        )
        in_maps.append(
            {
                "seg": np.ascontiguousarray(seg_b),
                "pad": np.ascontiguousarray(pad_ins_outs[b]).astype(
                    ml_dtypes.float8_e4m3
                ),
                "yf": yf,
                "tril": tril,
                "amin4": amin4,
                "amax4": amax4,
            }
        )
    return in_maps


def kernel(seg_outs: np.ndarray, pad_ins_outs: np.ndarray) -> np.ndarray:
    nc = _get_nc()
    in_maps = make_in_maps(seg_outs, pad_ins_outs)
    res = run_bass_kernel_spmd(nc, in_maps, list(range(B)))
    outs = [res.results[b]["out"].T for b in range(B)]  # [N, 3] each
    return np.stack(outs, axis=0).astype(np.float32)


if __name__ == "__main__":
    rng = np.random.default_rng(0)
    seg_outs = rng.standard_normal((B, H, W, 2), dtype=np.float32)
    pad_ins_outs = rng.random((B, N, H, W), dtype=np.float32)
    print(kernel(seg_outs, pad_ins_outs)[0, :4])
